# revision 1
# baseline (speedup 1.0000x reference)
"""GNN message-passing kernel for 8 trn2 NeuronCores (Bass/Tile).

Model (reference):
    msg  = relu(concat(x[src], x[dst], e_attr) @ W_msg + b_msg)   # [E, 30]
    x1   = segment_sum(msg, dst, N)                                # [N, 30]
    h    = relu(x1 @ W1 + b1)                                      # [N, 20]
    g    = segment_sum(h, batch, G)                                # [G, 20]
    out  = relu(g @ W2 + b2) @ W3 + b3                             # [G, 1]

Two-pass scheme:
  pass 1: each core computes P = node_attr @ W_src and Q = node_attr @
          W_dst for its 1/8 node range (output PQ [NPC, 64] bf16).
  host:   gathers PQ rows per edge endpoint (the "replicated node
          table" gather of the sharding strategy) and assembles one
          fused per-edge stream sM [128, slots]: rows 0-29 P[src],
          30-59 Q[dst], row 60 ones (bias), rows 64-127 e_attr^T.
  pass 2: edges are bucketed by dst (12544 nodes / 98 blocks of 128
          per core, each block statically padded to 2304 slots).  One
          matmul per 128-edge tile produces the messages (lhsT =
          stream tile, rhs = Wc with stacked I30/I30/b/W_e), relu on
          ACT, then a one-hot scatter matmul accumulates per-block
          node sums in PSUM.  Per-node MLP + graph pooling run per
          block; pooled per-graph partials are scattered into a
          [GPAD, 20] buffer, AllReduced across cores, and the tiny
          graph head runs redundantly on every core.
"""
import sys

if "/opt/trn_rl_repo" not in sys.path:
    sys.path.insert(0, "/opt/trn_rl_repo")

import numpy as np
import ml_dtypes

bf16 = ml_dtypes.bfloat16

# ---------------------------------------------------------------- config

class Cfg:
    N = 100000          # nodes
    E = 1600000         # edges
    D = 64              # feature dim
    G = 1000            # graphs
    DM = 30             # message dim
    NCORES = 8
    NPC = 12544         # nodes per core (98 * 128)
    NBLK = 98           # 128-node blocks per core
    BS = 2304           # edge slots per block (18 tiles)
    TPB = 18            # tiles per block
    CH = 9              # tiles per chunk
    NCH = 2             # chunks per block
    GSPAN = 192         # per-core graph window (incl. trash slots)
    GPAD = 1280         # padded global graph rows (1000 real + trash)

    @property
    def ES(self):
        return self.NBLK * self.BS

    @property
    def NT(self):
        return self.NBLK * self.TPB


FULL = Cfg()


def small_cfg():
    c = Cfg()
    c.N = 2048
    c.E = 8192
    c.G = 16
    c.NPC = 256
    c.NBLK = 2
    c.BS = 1536
    c.TPB = 12
    c.CH = 6
    c.NCH = 2
    c.GSPAN = 16
    c.GPAD = 384
    return c


# ---------------------------------------------------------------- pass 1

def build_pq_program(cfg):
    import concourse.bacc as bacc
    import concourse.mybir as mybir
    import concourse.tile as tile
    from contextlib import ExitStack

    f32, bft = mybir.dt.float32, mybir.dt.bfloat16
    COPY = mybir.ActivationFunctionType.Copy
    NPC = cfg.NPC
    NTL = NPC // 128

    nc = bacc.Bacc("TRN2", target_bir_lowering=False, debug=True)
    naT = nc.declare_dram_parameter("naT", [64, NPC], bft, isOutput=False)
    Wpq = nc.declare_dram_parameter("Wpq", [64, 64], bft, isOutput=False)
    PQ = nc.declare_dram_parameter("PQ", [128, NTL * 64], bft, isOutput=True)

    with tile.TileContext(nc) as tc, ExitStack() as xs:
        cp = xs.enter_context(tc.tile_pool(name="const", bufs=1))
        ps = xs.enter_context(tc.tile_pool(name="ps", bufs=2, space="PSUM"))
        naT_t = cp.tile([64, NPC], bft)
        nc.sync.dma_start(out=naT_t[:], in_=naT[:])
        Wpq_t = cp.tile([64, 64], bft)
        nc.sync.dma_start(out=Wpq_t[:], in_=Wpq[:])
        acc = cp.tile([128, NTL * 64], bft)
        for t in range(NTL):
            pq_ps = ps.tile([128, 64], f32, tag="pq")
            nc.tensor.matmul(pq_ps[:], lhsT=naT_t[:, t * 128:(t + 1) * 128],
                             rhs=Wpq_t[:], start=True, stop=True)
            nc.scalar.activation(acc[:, t * 64:(t + 1) * 64], pq_ps[:], COPY)
        nc.sync.dma_start(out=PQ[:], in_=acc[:])
    nc.finalize()
    return nc


# ---------------------------------------------------------------- pass 2

def build_main_program(cfg):
    import concourse.bass as bass
    import concourse.bacc as bacc
    import concourse.mybir as mybir
    import concourse.tile as tile
    from contextlib import ExitStack

    f32, bft, i32 = mybir.dt.float32, mybir.dt.bfloat16, mybir.dt.int32
    EQ = mybir.AluOpType.is_equal
    RELU = mybir.ActivationFunctionType.Relu
    COPY = mybir.ActivationFunctionType.Copy
    DM, CH, BS, TPB, NCH, NBLK = cfg.DM, cfg.CH, cfg.BS, cfg.TPB, cfg.NCH, cfg.NBLK
    GSPAN, GPAD = cfg.GSPAN, cfg.GPAD

    nc = bacc.Bacc("TRN2", target_bir_lowering=False, debug=True)

    NPAIR = NBLK // 2
    PADI = CH + (CH % 2)
    NCHT = NBLK * NCH
    HPAD = GPAD - 256
    sM = nc.declare_dram_parameter("sM", [NPAIR * 128, 2 * BS], bft, isOutput=False)
    dstoff = nc.declare_dram_parameter("dstoff", [128, NCHT * PADI], mybir.dt.int16, isOutput=False)
    batchrelF = nc.declare_dram_parameter("batchrelF", [128, NBLK], f32, isOutput=False)
    iotaG = nc.declare_dram_parameter("iotaG", [128, GSPAN], bft, isOutput=False)
    ident = nc.declare_dram_parameter("ident", [128, 128], f32, isOutput=False)
    Wc = nc.declare_dram_parameter("Wc", [128, DM], bft, isOutput=False)
    W1a = nc.declare_dram_parameter("W1a", [64, 20], bft, isOutput=False)
    W2a = nc.declare_dram_parameter("W2a", [64, 10], f32, isOutput=False)
    W3a = nc.declare_dram_parameter("W3a", [64, 1], f32, isOutput=False)
    gmapA = nc.declare_dram_parameter("gmapA", [128, 1], i32, isOutput=False)
    gmapB = nc.declare_dram_parameter("gmapB", [128, 1], i32, isOutput=False)
    out = nc.declare_dram_parameter("out", [1, GPAD], f32, isOutput=True)

    allin = nc.dram_tensor("allin", [GPAD, 20], f32)
    allout = nc.dram_tensor("allout", [GPAD, 20], f32)

    with tile.TileContext(nc) as tc, ExitStack() as xs:
        cp = xs.enter_context(tc.tile_pool(name="const", bufs=1))
        sMp = xs.enter_context(tc.tile_pool(name="sMp", bufs=3))
        ohp = xs.enter_context(tc.tile_pool(name="ohp", bufs=2))
        msgp = xs.enter_context(tc.tile_pool(name="msgp", bufs=2))
        smallp = xs.enter_context(tc.tile_pool(name="smallp", bufs=2))
        ps_msg = xs.enter_context(tc.tile_pool(name="ps_msg", bufs=2, space="PSUM"))
        ps_x = xs.enter_context(tc.tile_pool(name="ps_x", bufs=1, space="PSUM"))
        ps_g = xs.enter_context(tc.tile_pool(name="ps_g", bufs=1, space="PSUM"))

        # ---- constants
        dstoff_t = cp.tile([128, NCHT * PADI], mybir.dt.int16)
        nc.sync.dma_start(out=dstoff_t[:], in_=dstoff[:])
        ones10_t = cp.tile([128, PADI], bft)
        nc.vector.memset(ones10_t[:], 1.0)
        batchrel_t = cp.tile([128, NBLK], f32)
        nc.sync.dma_start(out=batchrel_t[:], in_=batchrelF[:])
        iotaG_t = cp.tile([128, GSPAN], bft)
        nc.sync.dma_start(out=iotaG_t[:], in_=iotaG[:])
        ident_t = cp.tile([128, 128], f32)
        nc.sync.dma_start(out=ident_t[:], in_=ident[:])
        Wc_t = cp.tile([128, DM], bft)
        nc.sync.dma_start(out=Wc_t[:], in_=Wc[:])
        W1a_t = cp.tile([64, 20], bft)
        nc.sync.dma_start(out=W1a_t[:], in_=W1a[:])
        W2a_t = cp.tile([64, 10], f32)
        nc.sync.dma_start(out=W2a_t[:], in_=W2a[:])
        W3a_t = cp.tile([64, 1], f32)
        nc.sync.dma_start(out=W3a_t[:], in_=W3a[:])
        gmapA_t = cp.tile([128, 1], i32)
        nc.sync.dma_start(out=gmapA_t[:], in_=gmapA[:])
        gmapB_t = cp.tile([128, 1], i32)
        nc.sync.dma_start(out=gmapB_t[:], in_=gmapB[:])

        # ---- zero the AllReduce input buffer
        zz = cp.tile([128, 20], f32)
        nc.vector.memset(zz[:], 0.0)
        for r in range(HPAD // 128):
            nc.sync.dma_start(out=allin[r * 128:(r + 1) * 128, :], in_=zz[:])

        # ---- pooled-graph accumulators (live across the whole main loop)
        GBW = max(GSPAN - 128, 0)
        gA_t = ps_g.tile([min(GSPAN, 128), 20], f32, tag="gA")
        gA_ps = gA_t[:]
        if GBW:
            gB_t = ps_g.tile([GBW, 20], f32, tag="gB")
            gB_ps = gB_t[:]
        else:
            gB_ps = None
        h_ps_tile = ps_g.tile([128, 20], f32, tag="hps")

        # ---- main loop over 128-node blocks (tails software-pipelined)
        sMpair_t = None
        pending = None          # (xT_ps, b) awaiting stage-2
        def tail_stage2(xT_ps, b):
            xTa_t = smallp.tile([64, 128], bft, tag="xta")
            nc.vector.memset(xTa_t[:, :], 0.0)
            nc.vector.memset(xTa_t[32:33, :], 1.0)
            nc.vector.tensor_copy(out=xTa_t[:DM, :], in_=xT_ps[:])
            nc.tensor.matmul(h_ps_tile[:], lhsT=xTa_t[:], rhs=W1a_t[:],
                             start=True, stop=True)
            h_t = smallp.tile([128, 20], bft, tag="h")
            nc.scalar.activation(h_t[:], h_ps_tile[:], RELU)
            ohg_t = smallp.tile([128, GSPAN], bft, tag="ohg")
            nc.vector.tensor_scalar(
                out=ohg_t[:], in0=iotaG_t[:], scalar1=batchrel_t[:, b:b + 1],
                scalar2=None, op0=EQ,
            )
            nc.tensor.matmul(gA_ps, lhsT=ohg_t[:, :min(GSPAN, 128)], rhs=h_t[:],
                             start=(b == 0), stop=(b == NBLK - 1))
            if gB_ps is not None:
                nc.tensor.matmul(gB_ps, lhsT=ohg_t[:, 128:GSPAN], rhs=h_t[:],
                                 start=(b == 0), stop=(b == NBLK - 1))

        for b in range(NBLK):
            if b % 2 == 0:
                j = b // 2
                sMpair_t = sMp.tile([128, 2 * BS], bft, tag="sM")
                eng = nc.sync if j % 2 == 0 else nc.scalar
                eng.dma_start(out=sMpair_t[:], in_=sM[j * 128:(j + 1) * 128, :])
                sM_t = sMpair_t[:, :BS]
            else:
                sM_t = sMpair_t[:, BS:]

            xblk_ps = ps_x.tile([128, DM], f32, tag="xblk")
            chunk_oh = []
            chunk_msg = []
            for ch in range(NCH):
                cix = (b * NCH + ch) * PADI
                msg_ps = ps_msg.tile([128, CH * DM], f32, tag="msgps")
                oh_t = ohp.tile([128, CH * 128], bft, tag="oh")
                nc.gpsimd.local_scatter(
                    out_ap=oh_t[:], data_ap=ones10_t[:],
                    idxs_ap=dstoff_t[:, cix:cix + PADI],
                    channels=128, num_elems=CH * 128, num_idxs=PADI,
                )
                for t in range(CH):
                    e0 = (ch * CH + t) * 128
                    nc.tensor.matmul(
                        msg_ps[:, t * DM:(t + 1) * DM],
                        lhsT=sM_t[:, e0:e0 + 128],
                        rhs=Wc_t[:],
                        start=True, stop=True,
                    )
                msg_t = msgp.tile([128, CH * DM], bft, tag="msg")
                nc.scalar.activation(msg_t[:], msg_ps[:], RELU)
                chunk_oh.append(oh_t)
                chunk_msg.append(msg_t)
            for ch in range(NCH):
                for t in range(CH):
                    gt = ch * CH + t
                    nc.tensor.matmul(
                        xblk_ps[:],
                        lhsT=chunk_oh[ch][:, t * 128:(t + 1) * 128],
                        rhs=chunk_msg[ch][:, t * DM:(t + 1) * DM],
                        start=(gt == 0), stop=(gt == TPB - 1),
                    )

            # tail stage 1 for this block: evict + transpose
            xs_t = smallp.tile([128, DM], f32, tag="xs")
            nc.vector.tensor_copy(out=xs_t[:], in_=xblk_ps[:])
            xT_ps = ps_msg.tile([DM, 128], f32, tag="xtps")
            nc.tensor.transpose(out=xT_ps[:], in_=xs_t[:], identity=ident_t[:])
            # tail stage 2 for the previous block
            if pending is not None:
                tail_stage2(*pending)
            pending = (xT_ps, b)
        tail_stage2(*pending)

        # ---- scatter per-core pooled partials into allin, then AllReduce
        pA_t = cp.tile([128, 20], f32)
        nc.vector.memset(pA_t[:, :], 0.0)
        nc.vector.tensor_copy(out=pA_t[:min(GSPAN, 128), :], in_=gA_ps[:])
        nc.gpsimd.indirect_dma_start(
            out=allin[:], out_offset=bass.IndirectOffsetOnAxis(ap=gmapA_t[:, :1], axis=0),
            in_=pA_t[:], in_offset=None,
        )
        if gB_ps is not None:
            pB_t = cp.tile([128, 20], f32)
            nc.vector.memset(pB_t[:, :], 0.0)
            nc.vector.tensor_copy(out=pB_t[:GBW, :], in_=gB_ps[:])
            nc.gpsimd.indirect_dma_start(
                out=allin[:], out_offset=bass.IndirectOffsetOnAxis(ap=gmapB_t[:, :1], axis=0),
                in_=pB_t[:], in_offset=None,
            )
        NR = HPAD // 128
        nc.gpsimd.collective_compute(
            "AllReduce", mybir.AluOpType.add,
            replica_groups=[list(range(cfg.NCORES))],
            ins=[allin[:HPAD, :]], outs=[allout[:HPAD, :]],
        )
        co_t = cp.tile([128, NR * 20], f32)
        nc.sync.dma_start(
            out=co_t[:].rearrange("p (a c) -> p a c", c=20),
            in_=allout[:HPAD, :].rearrange("(a p) c -> p a c", p=128),
        )

        # ---- graph head (redundant on every core)
        pta = cp.tile([64, HPAD], f32)
        nc.vector.memset(pta[:, :], 0.0)
        nc.vector.memset(pta[32:33, :], 1.0)
        for r in range(NR):
            tr_ps = ps_msg.tile([20, 128], f32, tag="xtps")
            nc.tensor.transpose(out=tr_ps[:], in_=co_t[:, r * 20:(r + 1) * 20],
                                identity=ident_t[:])
            nc.vector.tensor_copy(out=pta[:20, r * 128:(r + 1) * 128], in_=tr_ps[:])
        h2a = cp.tile([64, HPAD], f32)
        nc.vector.memset(h2a[:, :], 0.0)
        nc.vector.memset(h2a[32:33, :], 1.0)
        outsb = cp.tile([1, GPAD], f32)
        nc.vector.memset(outsb[:], 0.0)
        nchunks = (HPAD + 511) // 512
        for c in range(nchunks):
            lo = c * 512
            hi = min(HPAD, lo + 512)
            h2_ps = ps_msg.tile([10, hi - lo], f32, tag="msgps")
            nc.tensor.matmul(h2_ps[:], lhsT=W2a_t[:], rhs=pta[:, lo:hi],
                             start=True, stop=True)
            nc.scalar.activation(h2a[:10, lo:hi], h2_ps[:], RELU)
            o_ps = ps_msg.tile([1, hi - lo], f32, tag="xtps")
            nc.tensor.matmul(o_ps[:], lhsT=W3a_t[:], rhs=h2a[:, lo:hi],
                             start=True, stop=True)
            nc.vector.tensor_copy(out=outsb[:, lo:hi], in_=o_ps[:])
        nc.sync.dma_start(out=out[:], in_=outsb[:])

    nc.finalize()
    return nc


# ---------------------------------------------------------------- host prep

def host_plan(cfg, edge_index, batch):
    """Slot assignment + graph-window metadata (independent of features)."""
    N, E, G = cfg.N, cfg.E, cfg.G
    NPC, NBLK, BS, GSPAN = cfg.NPC, cfg.NBLK, cfg.BS, cfg.GSPAN

    src = np.asarray(edge_index[0]).astype(np.int64)
    dst = np.asarray(edge_index[1]).astype(np.int64)
    batch = np.asarray(batch).astype(np.int32)

    order = np.argsort(dst, kind="stable")
    src_s, dst_s = src[order], dst[order]
    blk = dst_s // 128
    nblk_tot = cfg.NCORES * NBLK
    cnt = np.bincount(blk, minlength=nblk_tot)
    assert cnt.max() <= BS, f"block overflow: {cnt.max()} > {BS}"
    starts = np.zeros(nblk_tot, np.int64)
    starts[1:] = np.cumsum(cnt)[:-1]
    rank = np.arange(E) - starts[blk]
    slot = blk * BS + rank

    NS_ALL = cfg.NCORES * cfg.ES
    drel = np.full(NS_ALL, -1.0, np.float32)
    drel[slot] = (dst_s % 128).astype(np.float32)

    g0 = np.zeros(cfg.NCORES, np.int32)
    batchrel = np.zeros([cfg.NCORES, NPC], np.float32)
    for c in range(cfg.NCORES):
        lo = c * NPC
        hi = min((c + 1) * NPC, N)
        g0[c] = batch[lo]
        rel = np.full(NPC, GSPAN - 1, np.float32)
        rel[:hi - lo] = (batch[lo:hi] - g0[c]).astype(np.float32)
        assert rel.max() <= GSPAN - 1
        batchrel[c] = rel

    return dict(order=order, src_s=src_s, dst_s=dst_s, slot=slot, drel=drel,
                g0=g0, batchrel=batchrel)


def host_prep_pq(cfg, node_attr, W_msg):
    naT = np.zeros([64, cfg.NCORES * cfg.NPC], bf16)
    naT[:, :cfg.N] = np.asarray(node_attr, np.float32).astype(bf16).T
    W_msg = np.asarray(W_msg, np.float32)
    Wpq = np.zeros([64, 64], np.float32)
    Wpq[:, 0:cfg.DM] = W_msg[0:64]
    Wpq[:, cfg.DM:2 * cfg.DM] = W_msg[64:128]
    Wpq = Wpq.astype(bf16)
    in_maps = []
    for c in range(cfg.NCORES):
        in_maps.append({
            "naT": np.ascontiguousarray(naT[:, c * cfg.NPC:(c + 1) * cfg.NPC]),
            "Wpq": Wpq,
        })
    return in_maps


def host_prep_main(cfg, plan, PQ_full, edge_attr, W_msg, b_msg,
                   W1, b1, W2, b2, W3, b3):
    G, DM = cfg.G, cfg.DM
    NBLK, BS, GSPAN, GPAD = cfg.NBLK, cfg.BS, cfg.GSPAN, cfg.GPAD
    ES = cfg.ES
    NS_ALL = cfg.NCORES * ES

    ea_bf = np.asarray(edge_attr, np.float32).astype(bf16)
    slot, order = plan["slot"], plan["order"]
    src_s, dst_s = plan["src_s"], plan["dst_s"]

    M = np.zeros([NS_ALL, 128], bf16)
    M[slot, 0:DM] = PQ_full[src_s, 0:DM]
    M[slot, DM:2 * DM] = PQ_full[dst_s, DM:2 * DM]
    M[:, 60] = bf16(1.0)
    M[slot, 64:128] = ea_bf[order]

    iotaG = np.broadcast_to(
        np.arange(GSPAN, dtype=np.float32), (128, GSPAN)).astype(bf16)
    ident = np.eye(128, dtype=np.float32)

    W_msg = np.asarray(W_msg, np.float32)
    Wcm = np.zeros([128, DM], np.float32)
    Wcm[0:DM] = np.eye(DM)
    Wcm[DM:2 * DM] = np.eye(DM)
    Wcm[60] = np.asarray(b_msg, np.float32)
    Wcm[64:128] = W_msg[128:192]
    Wcm = Wcm.astype(bf16)
    W1a = np.zeros([64, 20], np.float32)
    W1a[:DM] = np.asarray(W1, np.float32)
    W1a[32] = np.asarray(b1, np.float32)
    W1a = W1a.astype(bf16)
    W2a = np.zeros([64, 10], np.float32)
    W2a[:20] = np.asarray(W2, np.float32)
    W2a[32] = np.asarray(b2, np.float32)
    W3a = np.zeros([64, 1], np.float32)
    W3a[:10] = np.asarray(W3, np.float32)
    W3a[32] = np.asarray(b3, np.float32)

    # per-chunk local_scatter indices: value = t*128 + dstrel  (or -1 pad)
    CH = cfg.CH
    NCHB = cfg.NCH
    PADI = CH + (CH % 2)
    NCHT = NBLK * NCHB
    drel_all = plan["drel"]

    g0 = plan["g0"]
    in_maps = []
    for c in range(cfg.NCORES):
        Mc = M[c * ES:(c + 1) * ES].reshape(NBLK, BS, 128)
        # pair consecutive blocks side-by-side: [NBLK/2, 128, 2*BS]
        Mc = Mc.swapaxes(1, 2).reshape(NBLK // 2, 2, 128, BS)
        Mc = np.ascontiguousarray(Mc.swapaxes(1, 2)).reshape(
            (NBLK // 2) * 128, 2 * BS)
        # dstoff [128, NCHT*PADI]
        dr = drel_all[c * ES:(c + 1) * ES].reshape(cfg.NT, 128)  # [tile, p]
        dof = np.full([128, NCHT, PADI], -1, np.int16)
        tile_ids = np.arange(cfg.NT).reshape(NCHT, CH)
        vals = dr[tile_ids]                     # [NCHT, CH, 128]
        offs = np.where(
            vals >= 0,
            vals + (np.arange(CH)[None, :, None] * 128),
            -1,
        ).astype(np.int16)
        dof[:, :, :CH] = offs.transpose(2, 0, 1)
        dof = dof.reshape(128, NCHT * PADI)
        gmA = np.zeros([128, 1], np.int32)
        gmB = np.zeros([128, 1], np.int32)
        for i in range(128):
            gi = g0[c] + i
            gmA[i, 0] = gi if (gi < G and i < GSPAN) else GPAD - 256 + i
        for i in range(128):
            j = 128 + i
            gj = g0[c] + j
            gmB[i, 0] = gj if (gj < G and j < GSPAN) else GPAD - 128 + i
        assert gmA.max() < GPAD and gmB.max() < GPAD
        in_maps.append({
            "sM": Mc,
            "dstoff": np.ascontiguousarray(dof),
            "batchrelF": np.ascontiguousarray(
                plan["batchrel"][c].reshape(NBLK, 128).T),
            "iotaG": iotaG,
            "ident": ident,
            "Wc": Wcm, "W1a": W1a, "W2a": W2a, "W3a": W3a,
            "gmapA": gmA, "gmapB": gmB,
        })
    return in_maps


# ---------------------------------------------------------------- kernel

_CACHE = {}


def _get_programs(cfg):
    key = (cfg.N, cfg.E, cfg.BS)
    if key not in _CACHE:
        _CACHE[key] = (build_pq_program(cfg), build_main_program(cfg))
    return _CACHE[key]


last_exec_ns = None
last_exec_ns_pq = None


def _run(cfg, inputs):
    import os
    from concourse.bass_utils import run_bass_kernel_spmd

    global last_exec_ns, last_exec_ns_pq
    nc_pq, nc_main = _get_programs(cfg)
    trace = bool(os.environ.get("GNN_TRACE"))

    plan = host_plan(cfg, inputs["edge_index"], inputs["batch"])
    pq_maps = host_prep_pq(cfg, inputs["node_attr"], inputs["W_msg"])
    res1 = run_bass_kernel_spmd(nc_pq, pq_maps, list(range(cfg.NCORES)),
                                trace=trace)
    NTL = cfg.NPC // 128
    PQ_full = np.concatenate(
        [np.asarray(res1.results[c]["PQ"]).reshape(128, NTL, 64)
         .transpose(1, 0, 2).reshape(cfg.NPC, 64)
         for c in range(cfg.NCORES)], axis=0
    )
    last_exec_ns_pq = res1.exec_time_ns

    in_maps = host_prep_main(
        cfg, plan, PQ_full, inputs["edge_attr"], inputs["W_msg"],
        inputs["b_msg"], inputs["W1"], inputs["b1"], inputs["W2"],
        inputs["b2"], inputs["W3"], inputs["b3"],
    )
    res = run_bass_kernel_spmd(nc_main, in_maps, list(range(cfg.NCORES)),
                               trace=trace)
    last_exec_ns = res.exec_time_ns
    out = np.asarray(res.results[0]["out"]).reshape(-1)[:cfg.G]
    return out.reshape(cfg.G, 1).astype(np.float32)


def kernel(**inputs):
    return _run(FULL, inputs)



# revision 6
# speedup vs baseline: 2.0353x; 2.0353x over previous
"""GNN message-passing kernel for 8 trn2 NeuronCores (Bass/Tile).

Model (reference):
    msg  = relu(concat(x[src], x[dst], e_attr) @ W_msg + b_msg)   # [E, 30]
    x1   = segment_sum(msg, dst, N)                                # [N, 30]
    h    = relu(x1 @ W1 + b1)                                      # [N, 20]
    g    = segment_sum(h, batch, G)                                # [G, 20]
    out  = relu(g @ W2 + b2) @ W3 + b3                             # [G, 1]

Scheme (v2, "lane-aligned blocks"):
  Cores own whole graphs (G/8 graphs each -> contiguous node range, no
  cross-core node sharing, hence NO collectives).  Within a core, its
  nodes are sorted by in-degree (desc) and packed into blocks of 128
  "lanes".  Block b gets T_b = max in-block degree tiles; tile t holds
  edge #t of every lane (zero-padded columns produce msg == 0 because
  the bias is folded into the dst-side node projection).  The per-edge
  matmul output partition therefore IS the destination node: no
  one-hot scatter matmuls, no gpsimd one-hot builds.  relu runs on the
  scalar/gpsimd engines in 16-tile chunks; the per-node sum is a single
  DVE tensor_reduce per block.

  pass 1: P|Q' = [node_attr^T; 1]^T @ [W_src | W_dst; 0 | b]  (per-core
          node shard, streamed orientation: W stationary).
  host:   gathers P[src] + Q'[dst] (the "replicated node table" gather
          of the sharding strategy), sums them, and assembles the fp8
          per-edge stream sM [96, slots]: rows 0-29 P+Q', 30-93 e_attr.
  pass 2: per block: T_b msg matmuls (fp8 lhsT stream tile, rhs = Wc =
          [I30; W_e]) -> relu -> DVE reduce over tiles -> x1.  Per 4
          blocks: one PE transpose + one block-diagonal W1 matmul gives
          h for 512 nodes; per block one tiny one-hot (gpsimd
          local_scatter, 128 idxs) pools h into a per-core PSUM-resident
          gT [20, 128 graphs].  The graph head runs transposed (gT ->
          W2 -> W3) with no transposes and the core writes its own 125
          graphs; the host concatenates.
"""
import sys

if "/opt/trn_rl_repo" not in sys.path:
    sys.path.insert(0, "/opt/trn_rl_repo")

import numpy as np
import ml_dtypes

bf16 = ml_dtypes.bfloat16
f8 = ml_dtypes.float8_e4m3

NCORES = 8
DM = 30          # message dim
KS = 96          # stream rows (30 PQ + 64 e_attr + 2 pad)
CHT = 16         # msg tiles per relu chunk (16*30 f32 = 1920B < 2KB bank)
GRP = 4          # blocks per W1 group


# ---------------------------------------------------------------- pass 1

def build_pq_program(npc_pad):
    import concourse.bacc as bacc
    import concourse.mybir as mybir
    import concourse.tile as tile
    from contextlib import ExitStack

    f32, bft = mybir.dt.float32, mybir.dt.bfloat16
    COPY = mybir.ActivationFunctionType.Copy
    NCH = npc_pad // 512

    nc = bacc.Bacc("TRN2", target_bir_lowering=False, debug=True)
    naT = nc.declare_dram_parameter("naT", [65, npc_pad], bft, isOutput=False)
    Wpq = nc.declare_dram_parameter("Wpq", [65, 64], bft, isOutput=False)
    PQT = nc.declare_dram_parameter("PQT", [64, npc_pad], bft, isOutput=True)

    with tile.TileContext(nc) as tc, ExitStack() as xs:
        cp = xs.enter_context(tc.tile_pool(name="const", bufs=1))
        inp = xs.enter_context(tc.tile_pool(name="inp", bufs=6))
        outp = xs.enter_context(tc.tile_pool(name="outp", bufs=3))
        ps = xs.enter_context(tc.tile_pool(name="ps", bufs=3, space="PSUM"))

        Wpq_t = cp.tile([65, 64], bft)
        nc.sync.dma_start(out=Wpq_t[:], in_=Wpq[:])
        # input chunks of up to 5*512 columns, prefetched
        IN_C = 2560
        nin = (npc_pad + IN_C - 1) // IN_C
        in_tiles = []
        for i in range(nin):
            lo = i * IN_C
            hi = min(npc_pad, lo + IN_C)
            t = inp.tile([65, hi - lo], bft, tag="nat")
            eng = nc.sync if i % 2 == 0 else nc.gpsimd
            eng.dma_start(out=t[:], in_=naT[:, lo:hi])
            in_tiles.append((t, lo, hi))

        pend = None
        for c in range(NCH):
            lo = c * 512
            ti, tlo, thi = in_tiles[lo // IN_C]
            pq_ps = ps.tile([64, 512], f32, tag="pq")
            nc.tensor.matmul(pq_ps[:], lhsT=Wpq_t[:],
                             rhs=ti[:, lo - tlo:lo - tlo + 512],
                             start=True, stop=True)
            if c % 2 == 0:
                ot = outp.tile([64, 1024], bft, tag="pqt")
                nc.scalar.activation(ot[:, 0:512], pq_ps[:], COPY)
                pend = (ot, lo)
            else:
                ot, olo = pend
                nc.vector.tensor_copy(out=ot[:, 512:1024], in_=pq_ps[:])
                eng = nc.sync if c % 4 == 1 else nc.gpsimd
                eng.dma_start(out=PQT[:, olo:olo + 1024], in_=ot[:])
                pend = None
        if pend is not None:
            ot, olo = pend
            nc.sync.dma_start(out=PQT[:, olo:olo + 512], in_=ot[:, 0:512])
    nc.finalize()
    return nc


# ---------------------------------------------------------------- pass 2

def build_main_program(nblk, tb, ngmax):
    """nblk: padded block count (multiple of GRP); tb: per-block tile
    counts (uniform across cores); ngmax: graphs per core (<=128)."""
    import concourse.bacc as bacc
    import concourse.mybir as mybir
    import concourse.tile as tile
    from contextlib import ExitStack

    f32, bft, fp8 = mybir.dt.float32, mybir.dt.bfloat16, mybir.dt.float8e4
    RELU = mybir.ActivationFunctionType.Relu
    COPY = mybir.ActivationFunctionType.Copy
    MAX = mybir.AluOpType.max
    ADD = mybir.AluOpType.add
    AXX = mybir.AxisListType.X

    nt = sum(tb)
    off = np.zeros(len(tb) + 1, np.int64)
    off[1:] = np.cumsum(tb)

    nc = bacc.Bacc("TRN2", target_bir_lowering=False, debug=True)
    sM = nc.declare_dram_parameter("sM", [KS, nt * 128], fp8, isOutput=False)
    Wc = nc.declare_dram_parameter("Wc", [KS, DM], fp8, isOutput=False)
    W1d = nc.declare_dram_parameter("W1d", [GRP * DM + 1, GRP * 20], bft,
                                    isOutput=False)
    W2a = nc.declare_dram_parameter("W2a", [21, 16], f32, isOutput=False)
    W3a = nc.declare_dram_parameter("W3a", [11, 16], f32, isOutput=False)
    ident = nc.declare_dram_parameter("ident", [128, 128], f32, isOutput=False)
    gidx = nc.declare_dram_parameter("gidx", [128, nblk * 2], mybir.dt.int16,
                                     isOutput=False)
    out = nc.declare_dram_parameter("out", [1, 128], f32, isOutput=True)

    with tile.TileContext(nc) as tc, ExitStack() as xs:
        cp = xs.enter_context(tc.tile_pool(name="const", bufs=1))
        sMp = xs.enter_context(tc.tile_pool(name="sMp", bufs=3))
        msgp = xs.enter_context(tc.tile_pool(name="msgp", bufs=2))
        x14p = xs.enter_context(tc.tile_pool(name="x14p", bufs=2))
        xTap = xs.enter_context(tc.tile_pool(name="xTap", bufs=2))
        h4p = xs.enter_context(tc.tile_pool(name="h4p", bufs=2))
        ohp = xs.enter_context(tc.tile_pool(name="ohp", bufs=2))
        ps_m = xs.enter_context(tc.tile_pool(name="ps_m", bufs=2, space="PSUM"))
        ps_t = xs.enter_context(tc.tile_pool(name="ps_t", bufs=2, space="PSUM"))
        ps_h = xs.enter_context(tc.tile_pool(name="ps_h", bufs=2, space="PSUM"))
        ps_g = xs.enter_context(tc.tile_pool(name="ps_g", bufs=1, space="PSUM"))

        # ---- constants
        Wc_t = cp.tile([KS, DM], fp8)
        nc.sync.dma_start(out=Wc_t[:], in_=Wc[:])
        W1d_t = cp.tile([GRP * DM + 1, GRP * 20], bft)
        nc.gpsimd.dma_start(out=W1d_t[:], in_=W1d[:])
        W2a_t = cp.tile([21, 16], f32)
        nc.gpsimd.dma_start(out=W2a_t[:], in_=W2a[:])
        W3a_t = cp.tile([11, 16], f32)
        nc.gpsimd.dma_start(out=W3a_t[:], in_=W3a[:])
        ident_t = cp.tile([128, 128], f32)
        nc.scalar.dma_start(out=ident_t[:], in_=ident[:])
        gidx_t = cp.tile([128, nblk * 2], mybir.dt.int16)
        nc.scalar.dma_start(out=gidx_t[:], in_=gidx[:])
        ones2 = cp.tile([128, 2], bft)
        nc.vector.memset(ones2[:], 1.0)

        gT_ps = ps_g.tile([20, 128], f32, tag="gT")

        dmae = [nc.sync, nc.gpsimd, nc.scalar]
        ngrp = nblk // GRP
        relu_i = 0
        for g in range(ngrp):
            b0 = g * GRP
            gtiles = int(off[b0 + GRP] - off[b0])
            if gtiles > 0:
                smb = sMp.tile([KS, gtiles * 128], fp8, tag="sM")
                dmae[g % 3].dma_start(
                    out=smb[:],
                    in_=sM[:, int(off[b0]) * 128:int(off[b0 + GRP]) * 128])
            x14 = x14p.tile([128, GRP * DM + 1], f32, tag="x14")
            nc.vector.memset(x14[:, GRP * DM:GRP * DM + 1], 1.0)
            for i in range(GRP):
                b = b0 + i
                T = tb[b]
                if T == 0:
                    nc.vector.memset(x14[:, i * DM:(i + 1) * DM], 0.0)
                    continue
                loc = int(off[b] - off[b0]) * 128
                msgblk = msgp.tile([128, T * DM], bft, tag="msg")
                for c0 in range(0, T, CHT):
                    cn = min(CHT, T - c0)
                    mps = ps_m.tile([128, cn * DM], f32, tag="mps")
                    for t in range(cn):
                        e0 = loc + (c0 + t) * 128
                        nc.tensor.matmul(
                            mps[:, t * DM:(t + 1) * DM],
                            lhsT=smb[:, e0:e0 + 128],
                            rhs=Wc_t[:], start=True, stop=True)
                    dst = msgblk[:, c0 * DM:(c0 + cn) * DM]
                    if relu_i % 4 < 3:
                        nc.scalar.activation(dst, mps[:], RELU)
                    else:
                        nc.vector.tensor_scalar(
                            out=dst, in0=mps[:], scalar1=0.0, scalar2=None,
                            op0=MAX)
                    relu_i += 1
                nc.vector.tensor_reduce(
                    out=x14[:, i * DM:(i + 1) * DM],
                    in_=msgblk[:].rearrange("p (t d) -> p d t", d=DM),
                    axis=AXX, op=ADD)
            # ---- W1 for the 4 blocks
            xT_ps = ps_t.tile([GRP * DM + 1, 128], f32, tag="xT")
            nc.tensor.transpose(out=xT_ps[:], in_=x14[:], identity=ident_t[:])
            xTa = xTap.tile([GRP * DM + 1, 128], bft, tag="xTa")
            nc.scalar.activation(xTa[:], xT_ps[:], COPY)
            h4_ps = ps_h.tile([128, GRP * 20], f32, tag="h4")
            nc.tensor.matmul(h4_ps[:], lhsT=xTa[:], rhs=W1d_t[:],
                             start=True, stop=True)
            h4 = h4p.tile([128, GRP * 20], bft, tag="h4s")
            nc.scalar.activation(h4[:], h4_ps[:], RELU)
            # ---- pool each block into gT
            for i in range(GRP):
                b = b0 + i
                oh = ohp.tile([128, 128], bft, tag="oh")
                nc.gpsimd.local_scatter(
                    out_ap=oh[:], data_ap=ones2[:],
                    idxs_ap=gidx_t[:, b * 2:b * 2 + 2],
                    channels=128, num_elems=128, num_idxs=2)
                nc.tensor.matmul(gT_ps[:], lhsT=h4[:, i * 20:(i + 1) * 20],
                                 rhs=oh[:], start=(b == 0), stop=(b == nblk - 1))

        # ---- graph head (per-core, transposed; no collectives)
        gTa = cp.tile([21, 128], f32)
        nc.vector.memset(gTa[:], 1.0)
        nc.vector.tensor_copy(out=gTa[:20, :], in_=gT_ps[:])
        r_ps = ps_m.tile([16, 128], f32, tag="mps")
        nc.tensor.matmul(r_ps[:], lhsT=W2a_t[:], rhs=gTa[:], start=True,
                         stop=True)
        rTa = cp.tile([11, 128], f32)
        nc.vector.memset(rTa[:], 1.0)
        nc.scalar.activation(rTa[:10, :], r_ps[:10, :], RELU)
        o_ps = ps_h.tile([16, 128], f32, tag="h4")
        nc.tensor.matmul(o_ps[:], lhsT=W3a_t[:], rhs=rTa[:], start=True,
                         stop=True)
        outsb = cp.tile([1, 128], f32)
        nc.vector.tensor_copy(out=outsb[:], in_=o_ps[:1, :])
        nc.sync.dma_start(out=out[:], in_=outsb[:])

    nc.finalize()
    return nc


# ---------------------------------------------------------------- host prep

def host_plan(edge_index, batch, n_nodes, n_graphs):
    """Graph-aligned core ownership + degree-sorted lane blocks."""
    src = np.asarray(edge_index[0]).astype(np.int64)
    dst = np.asarray(edge_index[1]).astype(np.int64)
    batch = np.asarray(batch).astype(np.int64)
    N, G = n_nodes, n_graphs

    gcnt = np.bincount(batch, minlength=G)
    gstart = np.zeros(G + 1, np.int64)
    gstart[1:] = np.cumsum(gcnt)
    glo = [c * G // NCORES for c in range(NCORES + 1)]
    nlo = [int(gstart[glo[c]]) for c in range(NCORES + 1)]

    deg = np.bincount(dst, minlength=N)

    order_e = np.argsort(dst, kind="stable")
    dst_s = dst[order_e]
    src_s = src[order_e]
    # per-edge rank within its dst run
    run_start = np.zeros(len(dst_s), np.int64)
    newrun = np.r_[True, dst_s[1:] != dst_s[:-1]]
    idx = np.arange(len(dst_s))
    run_start = np.maximum.accumulate(np.where(newrun, idx, 0))
    rank = idx - run_start

    # per-core sorted lanes
    cores = []
    nblk_max = 0
    for c in range(NCORES):
        lo, hi = nlo[c], nlo[c + 1]
        dg = deg[lo:hi]
        order_n = np.argsort(-dg, kind="stable")
        lane_of = np.empty(hi - lo, np.int64)
        lane_of[order_n] = np.arange(hi - lo)
        nblk = (hi - lo + 127) // 128
        nblk_max = max(nblk_max, nblk)
        dgp = np.zeros(nblk * 128, np.int64)
        dgp[:hi - lo] = dg[order_n]
        tbc = dgp.reshape(nblk, 128).max(axis=1)
        cores.append(dict(lo=lo, hi=hi, order_n=order_n, lane_of=lane_of,
                          tbc=tbc, glo=glo[c], ghi=glo[c + 1]))

    nblk = ((nblk_max + GRP - 1) // GRP) * GRP
    tb = np.zeros(nblk, np.int64)
    for cd in cores:
        tb[:len(cd["tbc"])] = np.maximum(tb[:len(cd["tbc"])], cd["tbc"])
    off = np.zeros(nblk + 1, np.int64)
    off[1:] = np.cumsum(tb)

    ngmax = max(cd["ghi"] - cd["glo"] for cd in cores)
    assert ngmax <= 128

    return dict(cores=cores, nblk=nblk, tb=tb, off=off, nt=int(tb.sum()),
                order_e=order_e, dst_s=dst_s, src_s=src_s, rank=rank,
                deg=deg, ngmax=ngmax, batch=batch)


def host_prep_pq(plan, node_attr, npc_pad):
    na = np.asarray(node_attr, np.float32)
    in_maps = []
    for cd in plan["cores"]:
        lo, hi = cd["lo"], cd["hi"]
        naT = np.zeros([65, npc_pad], bf16)
        naT[0:64, :hi - lo] = na[lo:hi].T.astype(bf16)
        naT[64, :] = bf16(1.0)
        in_maps.append({"naT": naT})
    return in_maps


def make_weights(W_msg, b_msg, W1, b1, W2, b2, W3, b3):
    W_msg = np.asarray(W_msg, np.float32)
    Wpq = np.zeros([65, 64], np.float32)
    Wpq[0:64, 0:DM] = W_msg[0:64]
    Wpq[0:64, DM:2 * DM] = W_msg[64:128]
    Wpq[64, DM:2 * DM] = np.asarray(b_msg, np.float32)
    Wc = np.zeros([KS, DM], np.float32)
    Wc[0:DM] = np.eye(DM)
    Wc[DM:DM + 64] = W_msg[128:192]
    W1d = np.zeros([GRP * DM + 1, GRP * 20], np.float32)
    for i in range(GRP):
        W1d[i * DM:(i + 1) * DM, i * 20:(i + 1) * 20] = np.asarray(W1, np.float32)
        W1d[GRP * DM, i * 20:(i + 1) * 20] = np.asarray(b1, np.float32)
    W2a = np.zeros([21, 16], np.float32)
    W2a[0:20, 0:10] = np.asarray(W2, np.float32)
    W2a[20, 0:10] = np.asarray(b2, np.float32)
    W3a = np.zeros([11, 16], np.float32)
    W3a[0:10, 0:1] = np.asarray(W3, np.float32)
    W3a[10, 0:1] = np.asarray(b3, np.float32)
    return (Wpq.astype(bf16), Wc.astype(f8), W1d.astype(bf16), W2a, W3a)


def host_prep_main(plan, PQ_full, edge_attr, Wc8, W1d, W2a, W3a):
    """PQ_full: [60, N] f32 (P rows 0-29, Q' rows 30-59)."""
    nt, off, tb, nblk = plan["nt"], plan["off"], plan["tb"], plan["nblk"]
    src_s, dst_s, rank = plan["src_s"], plan["dst_s"], plan["rank"]
    batch = plan["batch"]

    PQsum = (PQ_full[0:30, src_s] + PQ_full[30:60, dst_s]).astype(f8)  # [30,E]
    ea8 = np.asarray(edge_attr, np.float32).astype(f8)  # [E, 64]
    ident = np.eye(128, dtype=np.float32)

    in_maps = []
    for cd in plan["cores"]:
        lo, hi = cd["lo"], cd["hi"]
        e0 = np.searchsorted(dst_s, lo)
        e1 = np.searchsorted(dst_s, hi)
        lane = cd["lane_of"][dst_s[e0:e1] - lo]
        blk = lane >> 7
        slot = (off[blk] + rank[e0:e1]) * 128 + (lane & 127)
        M = np.zeros([KS, nt * 128], f8)
        M[0:30, slot] = PQsum[:, e0:e1]
        M[30:94, slot] = ea8[plan["order_e"][e0:e1]].T
        gidx = np.full([128, nblk * 2], -1, np.int16)
        nn = hi - lo
        rel = (batch[lo:hi] - cd["glo"]).astype(np.int16)[cd["order_n"]]
        lanes = np.arange(nn)
        gidx[lanes & 127, (lanes >> 7) * 2] = rel
        in_maps.append({
            "sM": M, "Wc": Wc8, "W1d": W1d, "W2a": W2a, "W3a": W3a,
            "ident": ident, "gidx": gidx,
        })
    return in_maps


# ---------------------------------------------------------------- kernel

_CACHE = {}


def _get_pq_program(npc_pad):
    key = ("pq", npc_pad)
    if key not in _CACHE:
        _CACHE[key] = build_pq_program(npc_pad)
    return _CACHE[key]


def _get_main_program(nblk, tb, ngmax):
    key = ("main", nblk, tuple(tb), ngmax)
    if key not in _CACHE:
        _CACHE[key] = build_main_program(nblk, tuple(int(t) for t in tb), ngmax)
    return _CACHE[key]


last_exec_ns = None
last_exec_ns_pq = None


def kernel(edge_index, node_attr, edge_attr, batch,
           W_msg, b_msg, W1, b1, W2, b2, W3, b3):
    import os
    from concourse.bass_utils import run_bass_kernel_spmd

    global last_exec_ns, last_exec_ns_pq
    trace = bool(os.environ.get("GNN_TRACE"))

    N, D = node_attr.shape
    G = int(np.asarray(batch).max()) + 1 if batch is not None else 0
    # keep G robust: batch is sorted; use max+1 but at least NCORES
    G = max(G, NCORES)

    plan = host_plan(edge_index, batch, N, G)
    max_nc = max(cd["hi"] - cd["lo"] for cd in plan["cores"])
    npc_pad = ((max_nc + 511) // 512) * 512

    nc_pq = _get_pq_program(npc_pad)
    pq_maps = host_prep_pq(plan, node_attr, npc_pad)
    Wpq, Wc8, W1d, W2a, W3a = make_weights(W_msg, b_msg, W1, b1, W2, b2, W3, b3)
    for m in pq_maps:
        m["Wpq"] = Wpq
    res1 = run_bass_kernel_spmd(nc_pq, pq_maps, list(range(NCORES)),
                                trace=trace)
    last_exec_ns_pq = res1.exec_time_ns

    PQ_full = np.zeros([60, N], np.float32)
    for c, cd in enumerate(plan["cores"]):
        lo, hi = cd["lo"], cd["hi"]
        PQ_full[:, lo:hi] = np.asarray(
            res1.results[c]["PQT"]).astype(np.float32)[0:60, :hi - lo]

    nc_main = _get_main_program(plan["nblk"], plan["tb"], plan["ngmax"])
    in_maps = host_prep_main(plan, PQ_full, edge_attr, Wc8, W1d, W2a, W3a)
    res = run_bass_kernel_spmd(nc_main, in_maps, list(range(NCORES)),
                               trace=trace)
    last_exec_ns = res.exec_time_ns

    outv = np.zeros([G, 1], np.float32)
    for c, cd in enumerate(plan["cores"]):
        glo, ghi = cd["glo"], cd["ghi"]
        outv[glo:ghi, 0] = np.asarray(res.results[c]["out"])[0, :ghi - glo]
    return outv
